# revision 34
# baseline (speedup 1.0000x reference)
"""3-layer GCN (GCNConv + LayerNorm + ReLU x2, GCNConv) on 8 Trainium2 NeuronCores.

Strategy (node-partitioned, graph-parallel):
  - Nodes are sharded contiguously across the 8 cores (12500 each).
  - Layer tables are single bf16 (128 wide, 256B rows).  L1 table: u1 =
    (dinv*x)@W1 (x pre-scaled on host).  L2 table: u2 = h1@W2 where h1 carries
    the folded dinv.  L3 table: h2 itself (aggregation commutes with the dense
    transform: A@(h W3) = (A@h)@W3, applied after aggregation per dst block).
  - Tables are AllGather'ed per source chunk so every core holds all rows.
  - Aggregation per destination-block of 128 nodes: dma_gather per edge
    (int16 indices, 256B descriptors), segment-sum on the TensorEngine via
    one-hot indicator matmuls (indicators precomputed on host, streamed from
    HBM).  Gather columns are pooled per (group, chunk) so padding is paid
    once per group rather than once per block; boundary columns that span two
    blocks get one matmul per block with separately-masked indicators.
  - Self-loop edges never touch the SWDGE gather: each block's own rows are
    re-read from the local cc_in buffer with a single HWDGE DMA and added via
    an identity matmul that also opens the PSUM accumulation (start=True).
  - LayerNorm exploits scale invariance: with zero conv bias the per-row
    dinv[dst] factor cancels inside LN (up to an eps shift ~1e-4), so the
    post-aggregation scale is dropped; the NEXT layer's source-side dinv is
    folded into the ReLU activation's per-partition scale/bias.
  - Padding slots carry an all-zero indicator column so they contribute
    nothing.
"""

import numpy as np
import ml_dtypes

import concourse.bacc as bacc
import concourse.bass as bass
import concourse.mybir as mybir
from concourse.tile import TileContext
from concourse.vector_clock import ScopedClock
from concourse import bass_utils

F32 = mybir.dt.float32
BF16 = mybir.dt.bfloat16
F8 = mybir.dt.float8e4
I16 = mybir.dt.int16
LN_EPS = 1e-5


# ----------------------------------------------------------------------------
# TileContext drain patch: this walrus build rejects >1 sync wait on the
# kernel-tail drain CTRL instruction, so spread the global-clock waits over
# individual sync-engine nops before the drain.
# ----------------------------------------------------------------------------
def _patched_drain_and_barrier(self, tick_clock, wait_clock):
    nc = self.nc
    collector = nc.sync.nop(nofuse=True, hint="drain_wait_split")
    wait_clock.add_sem_waits(collector.ins, ScopedClock({None: tick_clock.global_clock}))
    si = collector.ins.sync_info
    if si is not None and si.on_wait and len(si.on_wait) > 1:
        waits = list(si.on_wait)
        del si.on_wait[1:]
        for w in waits[1:]:
            extra = nc.sync.nop(nofuse=True, hint="drain_wait_split")
            if extra.ins.sync_info is None:
                extra.ins.sync_info = mybir.SyncInfo(on_wait=[w], on_update=[])
            else:
                extra.ins.sync_info.on_wait.append(w)
    nc.sync.drain()
    nc.all_engine_barrier()
    assert self.sems is not None
    popped = nc._tile_sem_poison_stack.pop()
    assert popped is self._sem_poison
    nc.clear_and_free_semaphores(list(self.sems.allocated().values()))
    nc.all_engine_barrier()


TileContext._drain_and_barrier = _patched_drain_and_barrier


# ----------------------------------------------------------------------------
# Configuration
# ----------------------------------------------------------------------------
class Cfg:
    def __init__(self, N=100000, E=1600000, FIN=128, H=128, FOUT=64,
                 NCORES=8, CH=4, GRP=6, KMAXCOL=40):
        self.N, self.E = N, E
        self.FIN, self.H, self.FOUT = FIN, H, FOUT
        self.NC = NCORES
        self.CH = CH              # source chunks (gather table <= 32767 rows each)
        self.GRP = GRP            # dst blocks per group (PSUM tiles in flight)
        self.KMAXCOL = KMAXCOL    # max gather columns per dma_gather call
        self.KMAXIND = KMAXCOL + 16  # max indicator columns per call
        assert N % NCORES == 0
        self.OWN = N // NCORES
        assert self.OWN % CH == 0
        self.CR = self.OWN // CH          # rows per core per chunk
        self.CHROWS = self.CR * NCORES    # rows per chunk table
        assert self.CHROWS <= 32767
        self.NB = (self.OWN + 127) // 128  # dst blocks per core
        self.NG = (self.NB + GRP - 1) // GRP


# ----------------------------------------------------------------------------
# Host-side preprocessing
# ----------------------------------------------------------------------------
def _preprocess(cfg, edge_index):
    """Build shared (g,ch)-pooled column layout + per-core index/indicator arrays."""
    c = cfg
    src = np.asarray(edge_index[0]).astype(np.int64)
    dst = np.asarray(edge_index[1]).astype(np.int64)

    deg = np.bincount(dst, minlength=c.N).astype(np.float32) + 1.0
    dinv = (1.0 / np.sqrt(deg)).astype(np.float32)

    # The APPENDED PyG self-loops are handled separately (identity matmul from
    # local cc_in); original edges — including coincidental src==dst ones —
    # all go through the gather path.
    s, d = src, dst

    ks, rs = np.divmod(s, c.OWN)
    cs, ls = np.divmod(rs, c.CR)
    tloc = (ks * c.CR + ls).astype(np.int64)     # row within chunk table
    kd, rd = np.divmod(d, c.OWN)
    eb = rd // 128                               # dst block within core
    edl = rd % 128                               # dst slot within block
    gb = eb // c.GRP                             # dst group
    brel = eb - gb * c.GRP                       # block within group

    NGC = c.NG * c.CH
    gc = gb * c.CH + cs                          # (group, chunk) id

    # per-core per-(g,ch) counts -> shared column counts
    n_gc = np.zeros((c.NC, NGC), dtype=np.int64)
    for k in range(c.NC):
        n_gc[k] = np.bincount(gc[kd == k], minlength=NGC)
    cols_gc = (n_gc.max(axis=0) + 127) // 128    # [NG*CH] shared per-(g,ch) columns
    col_off = np.zeros(NGC + 1, dtype=np.int64)
    np.cumsum(cols_gc, out=col_off[1:])
    totcol = int(col_off[-1])

    # per-core slot assignment: edges of (g,ch) sorted by block, packed densely
    idx_all = np.zeros((c.NC, 16, totcol * 8), dtype=np.int16)
    slot_t = np.zeros(totcol * 128, dtype=np.int64)
    # per-column block membership union (shared across cores)
    col_has = np.zeros((totcol, c.GRP), dtype=bool)
    percore = []
    for k in range(c.NC):
        m = kd == k
        key = gc[m] * c.NB + eb[m]               # sort by (g, ch, block)
        so = np.argsort(key, kind="stable")
        gc_s = gc[m][so]
        t_s = tloc[m][so]
        br_s = brel[m][so]
        dl_s = edl[m][so]
        counts = np.bincount(gc_s, minlength=NGC)
        starts = np.zeros(NGC + 1, dtype=np.int64)
        np.cumsum(counts, out=starts[1:])
        within = np.arange(len(gc_s)) - starts[gc_s]
        colq = col_off[gc_s] + within // 128     # global gather column
        p = within % 128                          # slot within column
        pos = colq * 128 + p
        slot_t[:] = 0
        slot_t[pos] = t_s
        st = slot_t.reshape(totcol, 8, 16)
        idx_all[k] = st.transpose(2, 0, 1).reshape(16, totcol * 8).astype(np.int16)
        col_has[colq, br_s] = True
        percore.append((colq, p, br_s, dl_s))

    # matmul schedule: per (g, ch) calls split at KMAXCOL; per call the list of
    # (j, m_rel, brel) indicator matmuls in column order
    calls = [[[] for _ in range(c.CH)] for _ in range(c.NG)]
    mm_of_col = [[] for _ in range(totcol)]      # (m_global, brel) per column
    m_glob = 0
    last_mm_of_block = {}
    for g in range(c.NG):
        nblk = min((g + 1) * c.GRP, c.NB) - g * c.GRP
        for ch in range(c.CH):
            i = g * c.CH + ch
            q0g, q1g = int(col_off[i]), int(col_off[i + 1])
            q = q0g
            while q < q1g:
                ncols = min(c.KMAXCOL, q1g - q)
                m0 = m_glob
                mms = []
                for j in range(ncols):
                    qq = q + j
                    rels = np.nonzero(col_has[qq])[0]
                    if len(rels) == 0:
                        rels = [0]               # empty shared column: dummy matmul
                    for r in rels:
                        r = int(r)
                        assert r < nblk
                        b = g * c.GRP + r
                        mms.append([j, m_glob - m0, b, False])
                        last_mm_of_block[b] = (len(calls[g][ch]), g, ch, len(mms) - 1)
                        m_glob += 1
                assert m_glob - m0 <= c.KMAXIND, (m_glob - m0, c.KMAXIND)
                calls[g][ch].append(dict(q0=q, ncols=ncols, m0=m0, mms=mms))
                q += ncols
    M = m_glob
    for b, (ci, g, ch, mi) in last_mm_of_block.items():
        calls[g][ch][ci]["mms"][mi][3] = True

    # per-core indicator array [128, M*128] (fp8: 0/1 exact)
    ind_all = []
    one = np.float32(1.0).astype(ml_dtypes.float8_e4m3)
    # map (column, brel) -> m_global
    m_of_colb = {}
    for g in range(c.NG):
        for ch in range(c.CH):
            for call in calls[g][ch]:
                for (j, mrel, b, _stop) in call["mms"]:
                    m_of_colb[(call["q0"] + j, b - g * c.GRP)] = call["m0"] + mrel
    colb_m = np.full((totcol, c.GRP), -1, dtype=np.int64)
    for (qq, r), mg in m_of_colb.items():
        colb_m[qq, r] = mg
    for k in range(c.NC):
        colq, p, br_s, dl_s = percore[k]
        ind = np.zeros((128, M * 128), dtype=ml_dtypes.float8_e4m3)
        mg = colb_m[colq, br_s]
        assert (mg >= 0).all()
        ind.reshape(-1)[p * (M * 128) + mg * 128 + dl_s] = one
        ind_all.append(ind)

    meta = dict(totcol=totcol, M=M, calls=calls)
    return meta, dinv, idx_all, ind_all


def _block_row_segments(cfg, b):
    """cc_in row segments of block b split at chunk boundaries: (chunk, row0_in_chunk, slot0, n)."""
    c = cfg
    r0 = b * 128
    r1 = min(r0 + 128, c.OWN)
    segs = []
    r = r0
    while r < r1:
        ch = r // c.CR
        rend = min(r1, (ch + 1) * c.CR)
        segs.append((ch, r - ch * c.CR, r - r0, rend - r))
        r = rend
    return segs


# ----------------------------------------------------------------------------
# Program builder
# ----------------------------------------------------------------------------
def _build_program(cfg, meta, trivial):
    c = cfg
    totcol = meta["totcol"]
    M = meta["M"]
    calls = meta["calls"]
    NQ = 4  # SWDGE queues

    nc = bacc.Bacc("TRN2", target_bir_lowering=False, debug=False,
                   num_devices=c.NC, num_swdge_queues=NQ)

    x = nc.dram_tensor("x", (c.OWN, c.FIN), F32, kind="ExternalInput")
    w1 = nc.dram_tensor("w1", (c.FIN, c.H), F32, kind="ExternalInput")
    w2 = nc.dram_tensor("w2", (c.H, c.H), F32, kind="ExternalInput")
    w3 = nc.dram_tensor("w3", (c.H, c.FOUT), F32, kind="ExternalInput")
    idx_all = nc.dram_tensor("idx_all", (128, totcol * 8), I16, kind="ExternalInput")
    ind_all = nc.dram_tensor("ind_all", (128, M * 128), F8, kind="ExternalInput")
    ident = nc.dram_tensor("ident", (128, 128), F32, kind="ExternalInput")
    ident_bf = nc.dram_tensor("ident_bf", (128, 128), BF16, kind="ExternalInput")
    dinv_cols = nc.dram_tensor("dinv_cols", (128, c.NB), F32, kind="ExternalInput")
    # optional non-trivial affine params (replicated rows)
    aff = {}
    for nm, w in (("b1r", c.H), ("g1r", c.H), ("be1r", c.H),
                  ("b2r", c.H), ("g2r", c.H), ("be2r", c.H), ("b3r", c.FOUT)):
        if not trivial[nm]:
            aff[nm] = nc.dram_tensor(nm, (128, w), F32, kind="ExternalInput")
    y = nc.dram_tensor("y", (c.OWN, c.FOUT), F32, kind="ExternalOutput")

    # exchange buffers: single-bf16 tables, H wide for all three layers
    cc_in = [[nc.dram_tensor(f"cc_in{l}_{ch}", (c.CR, c.H), BF16)
              for ch in range(c.CH)] for l in range(1, 4)]
    cc_out = [[nc.dram_tensor(f"cc_out{l}_{ch}", (c.CHROWS, c.H),
                              BF16, addr_space="Shared")
               for ch in range(c.CH)] for l in range(1, 4)]

    # AG trigger points: after which group each chunk's contribution rows are done
    def ag_group(ch):
        last_row = (ch + 1) * c.CR - 1
        return (last_row // 128) // c.GRP

    with TileContext(nc) as tc:
        consts = tc.alloc_tile_pool(name="consts", bufs=1)
        xh = tc.alloc_tile_pool(name="xh", bufs=4)
        xtp = tc.alloc_tile_pool(name="xtp", bufs=3)
        uhl = tc.alloc_tile_pool(name="uhl", bufs=4)
        gp = tc.alloc_tile_pool(name="gp", bufs=8)
        sfp = tc.alloc_tile_pool(name="sfp", bufs=4)
        ip = tc.alloc_tile_pool(name="ip", bufs=3)
        lnp = tc.alloc_tile_pool(name="lnp", bufs=6)
        ps_t = tc.alloc_tile_pool(name="ps_t", bufs=1, space="PSUM")
        ps_d = tc.alloc_tile_pool(name="ps_d", bufs=1, space="PSUM")
        ps_a = tc.alloc_tile_pool(name="ps_a", bufs=6, space="PSUM")

        w1_sb = consts.tile([c.FIN, c.H], F32, tag="w1")
        w2_sb = consts.tile([c.H, c.H], F32, tag="w2")
        w3_sb = consts.tile([c.H, c.FOUT], F32, tag="w3")
        idx_sb = consts.tile([128, totcol * 8], I16, tag="idx")
        ident_sb = consts.tile([128, 128], F32, tag="ident")
        identbf_sb = consts.tile([128, 128], BF16, tag="identbf")
        dinv_sb = consts.tile([128, c.NB], F32, tag="dinv")
        eps_sb = consts.tile([128, 1], F32, tag="eps")
        nc.sync.dma_start(out=w1_sb[:], in_=w1[:])
        nc.sync.dma_start(out=w2_sb[:], in_=w2[:])
        nc.sync.dma_start(out=w3_sb[:], in_=w3[:])
        nc.sync.dma_start(out=idx_sb[:], in_=idx_all[:])
        nc.sync.dma_start(out=ident_sb[:], in_=ident[:])
        nc.sync.dma_start(out=identbf_sb[:], in_=ident_bf[:])
        nc.sync.dma_start(out=dinv_sb[:], in_=dinv_cols[:])
        nc.vector.memset(eps_sb[:], LN_EPS)
        aff_sb = {}
        for nm, t in aff.items():
            aff_sb[nm] = consts.tile(list(t.shape), F32, tag=nm, name=nm)
            nc.sync.dma_start(out=aff_sb[nm][:], in_=t[:])

        def write_rows(layer, b, tile, width):
            """DMA tile[:, :width] (bf16) to cc_in[layer] rows of block b."""
            for (ch, row0, slot0, nrows) in _block_row_segments(c, b):
                tgt = cc_in[layer - 1][ch]
                nc.sync.dma_start(out=tgt[row0:row0 + nrows, :width],
                                  in_=tile[slot0:slot0 + nrows, :width])

        def dense_to_table(h_sb, layer, b, pool_t=None, pool_d=None):
            """h_sb [128, H] -> u = h@W (dinv already folded into h); write bf16."""
            wname = (w1_sb, w2_sb, w3_sb)[layer - 1]
            fout = c.H if layer < 3 else c.FOUT
            tp = (pool_t or ps_t).tile([128, 128], F32, tag="tps")
            nc.tensor.transpose(out=tp[:], in_=h_sb[:], identity=ident_sb[:])
            hT = xtp.tile([128, 128], F32, tag="hT")
            nc.scalar.copy(out=hT[:], in_=tp[:])
            dp = (pool_d or ps_d).tile([128, c.H], F32, tag="dps")
            nc.tensor.matmul(dp[:, :fout], lhsT=hT[:], rhs=wname[:], start=True, stop=True)
            u = uhl.tile([128, c.H], BF16, tag="u")
            nc.scalar.copy(out=u[:, :fout], in_=dp[:, :fout])
            write_rows(layer, b, u, fout)

        def emit_ag(layer):
            done = [False] * c.CH

            def fire(ch):
                if done[ch]:
                    return
                done[ch] = True
                nc.gpsimd.collective_compute(
                    "AllGather", mybir.AluOpType.bypass,
                    replica_groups=[list(range(c.NC))],
                    ins=[cc_in[layer - 1][ch][:]],
                    outs=[cc_out[layer - 1][ch][:]],
                )

            def maybe(g):
                # non-final chunks trigger one group late: the collective
                # trigger's wait-for-previous-AG then has a full group of
                # runway, so it doesn't park the gpsimd queue (which would
                # stall gather descriptor generation).  The final chunk is
                # fired from inside the NEXT aggregation stream, just before
                # the first gather that reads it.
                for ch in range(c.CH - 1):
                    if not done[ch] and g >= ag_group(ch) + 1:
                        fire(ch)
            return maybe, fire

        # ---------------- layer 1 dense ----------------
        # x arrives pre-scaled by dinv on the host: u1 = x@W1.
        # Only chunk 0's AllGather fires from the dense loop; chunks 1-3 fire
        # from inside the layer-1 aggregation stream so the serialized AG chain
        # overlaps gather descriptor generation instead of blocking it.
        ag1_maybe, ag1_fire = emit_ag(1)
        for g in range(c.NG):
            for b in range(g * c.GRP, min((g + 1) * c.GRP, c.NB)):
                r0 = b * 128
                nrows = min(128, c.OWN - r0)
                xb = xh.tile([128, c.FIN], F32, tag="xh")
                if nrows < 128:
                    nc.vector.memset(xb[:], 0.0)
                nc.sync.dma_start(out=xb[:nrows, :], in_=x[r0:r0 + nrows, :])
                dense_to_table(xb, 1, b)
            if g >= ag_group(0):
                ag1_fire(0)

        # ---------------- aggregation layers ----------------
        def agg_layer(layer, pending, nxt):
            """Aggregate from cc_out[layer-1]; layer<3: LN+ReLU (dinv folded) then
            dense(layer+1) / table write; layer==3: swapped matmul -> @W3 -> y.
            pending: {ch: fire_fn} — this layer's table AGs still to trigger,
            emitted in group 0 right before the first gather reading them.
            nxt: (maybe, fire) for the next layer's table AGs, or None."""
            psum_tiles = {}
            ag_next = nxt[0] if nxt is not None else None
            for g in range(c.NG):
                blocks = range(g * c.GRP, min((g + 1) * c.GRP, c.NB))
                # self-loop contributions: local cc_in rows, identity matmul opens PSUM
                for b in blocks:
                    ps = ps_a.tile([128, c.H], F32, tag="aps", name=f"aps_{layer}_{b}")
                    psum_tiles[b] = ps
                    gs = sfp.tile([128, c.H], BF16, tag="gs")
                    for (ch, row0, slot0, nrows) in _block_row_segments(c, b):
                        nc.scalar.dma_start(out=gs[slot0:slot0 + nrows, :],
                                            in_=cc_in[layer - 1][ch][row0:row0 + nrows, :])
                    if layer < 3:
                        nc.tensor.matmul(ps[:], lhsT=identbf_sb[:], rhs=gs[:],
                                         start=True, stop=False)
                    else:
                        nc.tensor.matmul(ps[:], lhsT=gs[:], rhs=identbf_sb[:],
                                         start=True, stop=False)
                for ch in range(c.CH):
                    if g == 0 and ch in pending:
                        pending[ch](ch)
                    for call in calls[g][ch]:
                        q0, ncols, m0, mms = (call["q0"], call["ncols"],
                                              call["m0"], call["mms"])
                        nm = len(mms)
                        gt = gp.tile([128, c.KMAXCOL, c.H], BF16, tag="gt")
                        nc.gpsimd.dma_gather(
                            gt[:, :ncols, :], cc_out[layer - 1][ch][:],
                            idx_sb[:, q0 * 8:(q0 + ncols) * 8],
                            ncols * 128, ncols * 128, c.H,
                            single_packet=False, queue_num=ch % NQ)
                        ind = ip.tile([128, c.KMAXIND, 128], F8, tag="ind")
                        nc.scalar.dma_start(
                            out=ind[:, :nm, :],
                            in_=ind_all[:, m0 * 128:(m0 + nm) * 128].rearrange(
                                "p (n s) -> p n s", s=128))
                        for (j, mrel, b, stop) in mms:
                            if layer < 3:
                                nc.tensor.matmul(
                                    psum_tiles[b][:],
                                    lhsT=ind[:, mrel, :], rhs=gt[:, j, :],
                                    start=False, stop=stop)
                            else:
                                nc.tensor.matmul(
                                    psum_tiles[b][:],
                                    lhsT=gt[:, j, :], rhs=ind[:, mrel, :],
                                    start=False, stop=stop)
                # post-process completed blocks of this group
                for b in blocks:
                    ps = psum_tiles.pop(b)
                    if layer < 3:
                        bias_nm, gain_nm, beta_nm = (f"b{layer}r", f"g{layer}r", f"be{layer}r")
                        if bias_nm in aff_sb:
                            t = lnp.tile([128, c.H], F32, tag="t")
                            nc.scalar.activation(out=t[:], in_=ps[:],
                                                 func=mybir.ActivationFunctionType.Copy,
                                                 scale=dinv_sb[:, b:b + 1])
                            nc.vector.tensor_tensor(out=t[:], in0=t[:],
                                                    in1=aff_sb[bias_nm][:],
                                                    op=mybir.AluOpType.add)
                            z = t
                        else:
                            z = ps  # dinv scale cancels inside LN (bias == 0)
                        stats = lnp.tile([128, 6], F32, tag="stats")
                        nc.vector.bn_stats(out=stats[:], in_=z[:])
                        mv = lnp.tile([128, 2], F32, tag="mv")
                        nc.vector.bn_aggr(out=mv[:], in_=stats[:])
                        sd = lnp.tile([128, 1], F32, tag="sd")
                        nc.scalar.activation(out=sd[:], in_=mv[:, 1:2],
                                             func=mybir.ActivationFunctionType.Sqrt,
                                             bias=eps_sb[:])
                        rstd = lnp.tile([128, 1], F32, tag="rstd")
                        nc.vector.reciprocal(out=rstd[:], in_=sd[:])
                        h = xh.tile([128, c.H], F32, tag="xh")
                        if gain_nm in aff_sb or beta_nm in aff_sb:
                            nbias = lnp.tile([128, 1], F32, tag="nbias")
                            nc.vector.tensor_scalar(out=nbias[:], in0=mv[:, 0:1],
                                                    scalar1=rstd[:], scalar2=-1.0,
                                                    op0=mybir.AluOpType.mult,
                                                    op1=mybir.AluOpType.mult)
                            hn = lnp.tile([128, c.H], F32, tag="hn")
                            nc.scalar.activation(out=hn[:], in_=z[:],
                                                 func=mybir.ActivationFunctionType.Copy,
                                                 scale=rstd[:], bias=nbias[:])
                            if gain_nm in aff_sb:
                                nc.vector.tensor_tensor(out=hn[:], in0=hn[:],
                                                        in1=aff_sb[gain_nm][:],
                                                        op=mybir.AluOpType.mult)
                            if beta_nm in aff_sb:
                                nc.vector.tensor_tensor(out=hn[:], in0=hn[:],
                                                        in1=aff_sb[beta_nm][:],
                                                        op=mybir.AluOpType.add)
                            nc.scalar.activation(out=h[:], in_=hn[:],
                                                 func=mybir.ActivationFunctionType.Relu,
                                                 scale=dinv_sb[:, b:b + 1])
                        else:
                            # h = dinv_next * Relu((z - mu) * rstd):
                            # scale' = rstd*dinv, bias' = -mu*rstd*dinv
                            sc = lnp.tile([128, 1], F32, tag="sc")
                            nc.vector.tensor_tensor(out=sc[:], in0=rstd[:],
                                                    in1=dinv_sb[:, b:b + 1],
                                                    op=mybir.AluOpType.mult)
                            nb = lnp.tile([128, 1], F32, tag="nb")
                            nc.vector.tensor_scalar(out=nb[:], in0=mv[:, 0:1],
                                                    scalar1=sc[:], scalar2=-1.0,
                                                    op0=mybir.AluOpType.mult,
                                                    op1=mybir.AluOpType.mult)
                            nc.scalar.activation(out=h[:], in_=z[:],
                                                 func=mybir.ActivationFunctionType.Relu,
                                                 scale=sc[:], bias=nb[:])
                        if layer == 1:
                            dense_to_table(h, 2, b)
                        else:
                            # L3 table is h2 itself (bf16)
                            h2 = uhl.tile([128, c.H], BF16, tag="u")
                            nc.scalar.copy(out=h2[:], in_=h[:])
                            write_rows(3, b, h2, c.H)
                    else:
                        # psum holds agg3_T [feat, dst]; out = (agg3_T).T @ W3
                        aT = lnp.tile([128, c.H], F32, tag="aT")
                        nc.scalar.copy(out=aT[:], in_=ps[:])
                        op = ps_d.tile([128, c.H], F32, tag="dps")
                        nc.tensor.matmul(op[:, :c.FOUT], lhsT=aT[:], rhs=w3_sb[:],
                                         start=True, stop=True)
                        o = lnp.tile([128, c.FOUT], F32, tag="o")
                        nc.scalar.activation(out=o[:], in_=op[:, :c.FOUT],
                                             func=mybir.ActivationFunctionType.Copy,
                                             scale=dinv_sb[:, b:b + 1])
                        if "b3r" in aff_sb:
                            nc.vector.tensor_tensor(out=o[:], in0=o[:],
                                                    in1=aff_sb["b3r"][:],
                                                    op=mybir.AluOpType.add)
                        r0 = b * 128
                        nrows = min(128, c.OWN - r0)
                        nc.sync.dma_start(out=y[r0:r0 + nrows, :], in_=o[:nrows, :])
                if ag_next is not None:
                    ag_next(g)

        ag2 = emit_ag(2)
        ag3 = emit_ag(3)
        agg_layer(1, {1: ag1_fire, 2: ag1_fire, 3: ag1_fire}, ag2)
        agg_layer(2, {3: ag2[1]}, ag3)
        agg_layer(3, {3: ag3[1]}, None)

        for p in (ps_a, ps_d, ps_t, lnp, ip, sfp, gp, uhl, xtp, xh, consts):
            p.release()

    nc.compile()
    return nc


# ----------------------------------------------------------------------------
# Entry points
# ----------------------------------------------------------------------------
_cache = {}


def _prepare(cfg, inputs):
    c = cfg
    key = hash((np.asarray(inputs["edge_index"]).tobytes(),
                np.asarray(inputs["x"]).tobytes()))
    if key in _cache:
        return _cache[key]

    meta, dinv, idx_all, ind_all = _preprocess(c, inputs["edge_index"])

    trivial = {
        "b1r": not np.any(inputs["b1"]), "g1r": bool(np.all(inputs["g1"] == 1.0)),
        "be1r": not np.any(inputs["be1"]), "b2r": not np.any(inputs["b2"]),
        "g2r": bool(np.all(inputs["g2"] == 1.0)), "be2r": not np.any(inputs["be2"]),
        "b3r": not np.any(inputs["b3"]),
    }
    nc = _build_program(c, meta, trivial)

    shared = {
        "w1": np.asarray(inputs["W1"], dtype=np.float32),
        "w2": np.asarray(inputs["W2"], dtype=np.float32),
        "w3": np.asarray(inputs["W3"], dtype=np.float32),
        "ident": np.eye(128, dtype=np.float32),
        "ident_bf": np.eye(128, dtype=np.float32).astype(ml_dtypes.bfloat16),
    }
    for nm, src in (("b1r", "b1"), ("g1r", "g1"), ("be1r", "be1"), ("b2r", "b2"),
                    ("g2r", "g2"), ("be2r", "be2"), ("b3r", "b3")):
        if not trivial[nm]:
            shared[nm] = np.asarray(inputs[src], dtype=np.float32)[None, :].repeat(128, 0).copy()

    x_np = np.asarray(inputs["x"], dtype=np.float32)
    in_maps = []
    for k in range(c.NC):
        dv = dinv[k * c.OWN:(k + 1) * c.OWN]
        dcols = np.zeros((128, c.NB), dtype=np.float32)
        npad = c.NB * 128 - c.OWN
        dvp = np.concatenate([dv, np.ones(npad, dtype=np.float32)])
        dcols[:, :] = dvp.reshape(c.NB, 128).T
        m = dict(shared)
        # pre-scale x rows by dinv on the host: u1 = (dinv*x)@W1
        m["x"] = np.ascontiguousarray(x_np[k * c.OWN:(k + 1) * c.OWN]) * dv[:, None]
        m["idx_all"] = np.tile(idx_all[k], (8, 1))
        m["ind_all"] = ind_all[k]
        m["dinv_cols"] = dcols
        in_maps.append(m)

    _cache[key] = (nc, in_maps)
    return nc, in_maps


def _run(cfg, inputs, trace=False):
    nc, in_maps = _prepare(cfg, inputs)
    res = bass_utils.run_bass_kernel_spmd(
        nc, in_maps, core_ids=list(range(cfg.NC)), trace=trace)
    out = np.concatenate([res.results[k]["y"] for k in range(cfg.NC)], axis=0)
    return out, res


def kernel(**inputs):
    cfg = Cfg()
    out, _ = _run(cfg, inputs)
    return out


# revision 39
# speedup vs baseline: 1.0639x; 1.0639x over previous
"""3-layer GCN (GCNConv + LayerNorm + ReLU x2, GCNConv) on 8 Trainium2 NeuronCores.

Strategy (node-partitioned, graph-parallel):
  - Nodes are sharded contiguously across the 8 cores (12500 each).
  - Layer tables are single bf16 (128 wide, 256B rows).  L1 table: u1 =
    (dinv*x)@W1 (x pre-scaled on host).  L2 table: u2 = h1@W2 where h1 carries
    the folded dinv.  L3 table: h2 itself (aggregation commutes with the dense
    transform: A@(h W3) = (A@h)@W3, applied after aggregation per dst block).
  - Tables are AllGather'ed per source chunk so every core holds all rows.
  - Aggregation per destination-block of 128 nodes: dma_gather per edge
    (int16 indices, 256B descriptors), segment-sum on the TensorEngine via
    one-hot indicator matmuls (indicators precomputed on host, streamed from
    HBM).  Gather columns are pooled per (group, chunk) so padding is paid
    once per group rather than once per block; boundary columns that span two
    blocks get one matmul per block with separately-masked indicators.
  - Self-loop edges never touch the SWDGE gather: each block's own rows are
    re-read from the local cc_in buffer with a single HWDGE DMA and added via
    an identity matmul that also opens the PSUM accumulation (start=True).
  - LayerNorm exploits scale invariance: with zero conv bias the per-row
    dinv[dst] factor cancels inside LN (up to an eps shift ~1e-4), so the
    post-aggregation scale is dropped; the NEXT layer's source-side dinv is
    folded into the ReLU activation's per-partition scale/bias.
  - Padding slots carry an all-zero indicator column so they contribute
    nothing.
"""

import numpy as np
import ml_dtypes

import concourse.bacc as bacc
import concourse.bass as bass
import concourse.mybir as mybir
from concourse.tile import TileContext
from concourse.vector_clock import ScopedClock
from concourse import bass_utils

F32 = mybir.dt.float32
BF16 = mybir.dt.bfloat16
F8 = mybir.dt.float8e4
I16 = mybir.dt.int16
LN_EPS = 1e-5


# ----------------------------------------------------------------------------
# TileContext drain patch: this walrus build rejects >1 sync wait on the
# kernel-tail drain CTRL instruction, so spread the global-clock waits over
# individual sync-engine nops before the drain.
# ----------------------------------------------------------------------------
def _patched_drain_and_barrier(self, tick_clock, wait_clock):
    nc = self.nc
    collector = nc.sync.nop(nofuse=True, hint="drain_wait_split")
    wait_clock.add_sem_waits(collector.ins, ScopedClock({None: tick_clock.global_clock}))
    si = collector.ins.sync_info
    if si is not None and si.on_wait and len(si.on_wait) > 1:
        waits = list(si.on_wait)
        del si.on_wait[1:]
        for w in waits[1:]:
            extra = nc.sync.nop(nofuse=True, hint="drain_wait_split")
            if extra.ins.sync_info is None:
                extra.ins.sync_info = mybir.SyncInfo(on_wait=[w], on_update=[])
            else:
                extra.ins.sync_info.on_wait.append(w)
    nc.sync.drain()
    nc.all_engine_barrier()
    assert self.sems is not None
    popped = nc._tile_sem_poison_stack.pop()
    assert popped is self._sem_poison
    nc.clear_and_free_semaphores(list(self.sems.allocated().values()))
    nc.all_engine_barrier()


TileContext._drain_and_barrier = _patched_drain_and_barrier


# ----------------------------------------------------------------------------
# Configuration
# ----------------------------------------------------------------------------
class Cfg:
    def __init__(self, N=100000, E=1600000, FIN=128, H=128, FOUT=64,
                 NCORES=8, CH=4, GRP=6, KMAXCOL=40):
        self.N, self.E = N, E
        self.FIN, self.H, self.FOUT = FIN, H, FOUT
        self.NC = NCORES
        self.CH = CH              # source chunks (gather table <= 32767 rows each)
        self.GRP = GRP            # dst blocks per group (PSUM tiles in flight)
        self.KMAXCOL = KMAXCOL    # max gather columns per dma_gather call
        self.KMAXIND = KMAXCOL + 16  # max indicator columns per call
        assert N % NCORES == 0
        self.OWN = N // NCORES
        assert self.OWN % CH == 0
        self.CR = self.OWN // CH          # rows per core per chunk
        self.CHROWS = self.CR * NCORES    # rows per chunk table
        assert self.CHROWS <= 32767
        self.NB = (self.OWN + 127) // 128  # dst blocks per core
        self.NG = (self.NB + GRP - 1) // GRP


# ----------------------------------------------------------------------------
# Host-side preprocessing
# ----------------------------------------------------------------------------
def _preprocess(cfg, edge_index):
    """Build shared (g,ch)-pooled column layout + per-core index/indicator arrays."""
    c = cfg
    src = np.asarray(edge_index[0]).astype(np.int64)
    dst = np.asarray(edge_index[1]).astype(np.int64)

    deg = np.bincount(dst, minlength=c.N).astype(np.float32) + 1.0
    dinv = (1.0 / np.sqrt(deg)).astype(np.float32)

    # The APPENDED PyG self-loops are handled separately (identity matmul from
    # local cc_in); original edges — including coincidental src==dst ones —
    # all go through the gather path.
    s, d = src, dst

    ks, rs = np.divmod(s, c.OWN)
    cs, ls = np.divmod(rs, c.CR)
    tloc = (ks * c.CR + ls).astype(np.int64)     # row within chunk table
    kd, rd = np.divmod(d, c.OWN)
    eb = rd // 128                               # dst block within core
    edl = rd % 128                               # dst slot within block
    gb = eb // c.GRP                             # dst group
    brel = eb - gb * c.GRP                       # block within group

    NGC = c.NG * c.CH
    gc = gb * c.CH + cs                          # (group, chunk) id

    # per-core per-(g,ch) counts -> shared column counts
    n_gc = np.zeros((c.NC, NGC), dtype=np.int64)
    for k in range(c.NC):
        n_gc[k] = np.bincount(gc[kd == k], minlength=NGC)
    cols_gc = (n_gc.max(axis=0) + 127) // 128    # [NG*CH] shared per-(g,ch) columns
    col_off = np.zeros(NGC + 1, dtype=np.int64)
    np.cumsum(cols_gc, out=col_off[1:])
    totcol = int(col_off[-1])

    # per-core slot assignment: edges of (g,ch) sorted by block, packed densely
    idx_all = np.zeros((c.NC, 16, totcol * 8), dtype=np.int16)
    slot_t = np.zeros(totcol * 128, dtype=np.int64)
    # per-column block membership union (shared across cores)
    col_has = np.zeros((totcol, c.GRP), dtype=bool)
    percore = []
    for k in range(c.NC):
        m = kd == k
        key = gc[m] * c.NB + eb[m]               # sort by (g, ch, block)
        so = np.argsort(key, kind="stable")
        gc_s = gc[m][so]
        t_s = tloc[m][so]
        br_s = brel[m][so]
        dl_s = edl[m][so]
        counts = np.bincount(gc_s, minlength=NGC)
        starts = np.zeros(NGC + 1, dtype=np.int64)
        np.cumsum(counts, out=starts[1:])
        within = np.arange(len(gc_s)) - starts[gc_s]
        colq = col_off[gc_s] + within // 128     # global gather column
        p = within % 128                          # slot within column
        pos = colq * 128 + p
        slot_t[:] = 0
        slot_t[pos] = t_s
        st = slot_t.reshape(totcol, 8, 16)
        idx_all[k] = st.transpose(2, 0, 1).reshape(16, totcol * 8).astype(np.int16)
        col_has[colq, br_s] = True
        percore.append((colq, p, br_s, dl_s))

    # matmul schedule: per (g, ch) calls split at KMAXCOL; per call the list of
    # (j, m_rel, brel) indicator matmuls in column order
    calls = [[[] for _ in range(c.CH)] for _ in range(c.NG)]
    mm_of_col = [[] for _ in range(totcol)]      # (m_global, brel) per column
    m_glob = 0
    last_mm_of_block = {}
    for g in range(c.NG):
        nblk = min((g + 1) * c.GRP, c.NB) - g * c.GRP
        for ch in range(c.CH):
            i = g * c.CH + ch
            q0g, q1g = int(col_off[i]), int(col_off[i + 1])
            q = q0g
            while q < q1g:
                ncols = min(c.KMAXCOL, q1g - q)
                m0 = m_glob
                mms = []
                for j in range(ncols):
                    qq = q + j
                    rels = np.nonzero(col_has[qq])[0]
                    if len(rels) == 0:
                        rels = [0]               # empty shared column: dummy matmul
                    for r in rels:
                        r = int(r)
                        assert r < nblk
                        b = g * c.GRP + r
                        mms.append([j, m_glob - m0, b, False])
                        last_mm_of_block[b] = (len(calls[g][ch]), g, ch, len(mms) - 1)
                        m_glob += 1
                assert m_glob - m0 <= c.KMAXIND, (m_glob - m0, c.KMAXIND)
                calls[g][ch].append(dict(q0=q, ncols=ncols, m0=m0, mms=mms))
                q += ncols
    M = m_glob
    for b, (ci, g, ch, mi) in last_mm_of_block.items():
        calls[g][ch][ci]["mms"][mi][3] = True

    # per-core indicator array [128, M*128] (fp8: 0/1 exact)
    ind_all = []
    one = np.float32(1.0).astype(ml_dtypes.float8_e4m3)
    # map (column, brel) -> m_global
    m_of_colb = {}
    for g in range(c.NG):
        for ch in range(c.CH):
            for call in calls[g][ch]:
                for (j, mrel, b, _stop) in call["mms"]:
                    m_of_colb[(call["q0"] + j, b - g * c.GRP)] = call["m0"] + mrel
    colb_m = np.full((totcol, c.GRP), -1, dtype=np.int64)
    for (qq, r), mg in m_of_colb.items():
        colb_m[qq, r] = mg
    for k in range(c.NC):
        colq, p, br_s, dl_s = percore[k]
        ind = np.zeros((128, M * 128), dtype=ml_dtypes.float8_e4m3)
        mg = colb_m[colq, br_s]
        assert (mg >= 0).all()
        ind.reshape(-1)[p * (M * 128) + mg * 128 + dl_s] = one
        ind_all.append(ind)

    meta = dict(totcol=totcol, M=M, calls=calls)
    return meta, dinv, idx_all, ind_all


def _block_row_segments(cfg, b):
    """cc_in row segments of block b split at chunk boundaries: (chunk, row0_in_chunk, slot0, n)."""
    c = cfg
    r0 = b * 128
    r1 = min(r0 + 128, c.OWN)
    segs = []
    r = r0
    while r < r1:
        ch = r // c.CR
        rend = min(r1, (ch + 1) * c.CR)
        segs.append((ch, r - ch * c.CR, r - r0, rend - r))
        r = rend
    return segs


# ----------------------------------------------------------------------------
# Program builder
# ----------------------------------------------------------------------------
def _build_program(cfg, meta, trivial):
    c = cfg
    totcol = meta["totcol"]
    M = meta["M"]
    calls = meta["calls"]
    NQ = 4  # SWDGE queues

    nc = bacc.Bacc("TRN2", target_bir_lowering=False, debug=False,
                   num_devices=c.NC, num_swdge_queues=NQ)

    xT = nc.dram_tensor("xT", (c.FIN, c.OWN), F32, kind="ExternalInput")
    w1 = nc.dram_tensor("w1", (c.FIN, c.H), F32, kind="ExternalInput")
    w2 = nc.dram_tensor("w2", (c.H, c.H), F32, kind="ExternalInput")
    w3 = nc.dram_tensor("w3", (c.H, c.FOUT), F32, kind="ExternalInput")
    idx_all = nc.dram_tensor("idx_all", (128, totcol * 8), I16, kind="ExternalInput")
    ind_all = nc.dram_tensor("ind_all", (128, M * 128), F8, kind="ExternalInput")
    ident = nc.dram_tensor("ident", (128, 128), F32, kind="ExternalInput")
    ident_bf = nc.dram_tensor("ident_bf", (128, 128), BF16, kind="ExternalInput")
    dinv_cols = nc.dram_tensor("dinv_cols", (128, c.NB), F32, kind="ExternalInput")
    # optional non-trivial affine params (replicated rows)
    aff = {}
    for nm, w in (("b1r", c.H), ("g1r", c.H), ("be1r", c.H),
                  ("b2r", c.H), ("g2r", c.H), ("be2r", c.H), ("b3r", c.FOUT)):
        if not trivial[nm]:
            aff[nm] = nc.dram_tensor(nm, (128, w), F32, kind="ExternalInput")
    y = nc.dram_tensor("y", (c.OWN, c.FOUT), F32, kind="ExternalOutput")

    # exchange buffers: single-bf16 tables, H wide for all three layers
    cc_in = [[nc.dram_tensor(f"cc_in{l}_{ch}", (c.CR, c.H), BF16)
              for ch in range(c.CH)] for l in range(1, 4)]
    cc_out = [[nc.dram_tensor(f"cc_out{l}_{ch}", (c.CHROWS, c.H),
                              BF16, addr_space="Shared")
               for ch in range(c.CH)] for l in range(1, 4)]

    # AG trigger points: after which group each chunk's contribution rows are done
    def ag_group(ch):
        last_row = (ch + 1) * c.CR - 1
        return (last_row // 128) // c.GRP

    with TileContext(nc) as tc:
        consts = tc.alloc_tile_pool(name="consts", bufs=1)
        xh = tc.alloc_tile_pool(name="xh", bufs=6)
        xtp = tc.alloc_tile_pool(name="xtp", bufs=4)
        uhl = tc.alloc_tile_pool(name="uhl", bufs=6)
        gp = tc.alloc_tile_pool(name="gp", bufs=8)
        sfp = tc.alloc_tile_pool(name="sfp", bufs=6)
        ip = tc.alloc_tile_pool(name="ip", bufs=4)
        lnp = tc.alloc_tile_pool(name="lnp", bufs=8)
        ps_t = tc.alloc_tile_pool(name="ps_t", bufs=1, space="PSUM")
        ps_d = tc.alloc_tile_pool(name="ps_d", bufs=1, space="PSUM")
        ps_a = tc.alloc_tile_pool(name="ps_a", bufs=6, space="PSUM")

        w1_sb = consts.tile([c.FIN, c.H], F32, tag="w1")
        w2_sb = consts.tile([c.H, c.H], F32, tag="w2")
        w3_sb = consts.tile([c.H, c.FOUT], F32, tag="w3")
        idx_sb = consts.tile([128, totcol * 8], I16, tag="idx")
        ident_sb = consts.tile([128, 128], F32, tag="ident")
        identbf_sb = consts.tile([128, 128], BF16, tag="identbf")
        dinv_sb = consts.tile([128, c.NB], F32, tag="dinv")
        eps_sb = consts.tile([128, 1], F32, tag="eps")
        nc.sync.dma_start(out=w1_sb[:], in_=w1[:])
        nc.sync.dma_start(out=w2_sb[:], in_=w2[:])
        nc.sync.dma_start(out=w3_sb[:], in_=w3[:])
        nc.sync.dma_start(out=idx_sb[:], in_=idx_all[:])
        nc.sync.dma_start(out=ident_sb[:], in_=ident[:])
        nc.sync.dma_start(out=identbf_sb[:], in_=ident_bf[:])
        nc.sync.dma_start(out=dinv_sb[:], in_=dinv_cols[:])
        nc.vector.memset(eps_sb[:], LN_EPS)
        aff_sb = {}
        for nm, t in aff.items():
            aff_sb[nm] = consts.tile(list(t.shape), F32, tag=nm, name=nm)
            nc.sync.dma_start(out=aff_sb[nm][:], in_=t[:])

        def write_rows(layer, b, tile, width):
            """DMA tile[:, :width] (bf16) to cc_in[layer] rows of block b."""
            for (ch, row0, slot0, nrows) in _block_row_segments(c, b):
                tgt = cc_in[layer - 1][ch]
                nc.sync.dma_start(out=tgt[row0:row0 + nrows, :width],
                                  in_=tile[slot0:slot0 + nrows, :width])

        def dense_to_table(h_sb, layer, b, pool_t=None, pool_d=None):
            """h_sb [128, H] -> u = h@W (dinv already folded into h); write bf16."""
            wname = (w1_sb, w2_sb, w3_sb)[layer - 1]
            fout = c.H if layer < 3 else c.FOUT
            tp = (pool_t or ps_t).tile([128, 128], F32, tag="tps")
            nc.tensor.transpose(out=tp[:], in_=h_sb[:], identity=ident_sb[:])
            hT = xtp.tile([128, 128], F32, tag="hT")
            nc.scalar.copy(out=hT[:], in_=tp[:])
            dp = (pool_d or ps_d).tile([128, c.H], F32, tag="dps")
            nc.tensor.matmul(dp[:, :fout], lhsT=hT[:], rhs=wname[:], start=True, stop=True)
            u = uhl.tile([128, c.H], BF16, tag="u")
            nc.scalar.copy(out=u[:, :fout], in_=dp[:, :fout])
            write_rows(layer, b, u, fout)

        def emit_ag(layer):
            done = [False] * c.CH

            def maybe(g):
                for ch in range(c.CH):
                    # non-final chunks trigger one group late: the collective
                    # trigger's wait-for-previous-AG then has a full group of
                    # runway, so it doesn't park the gpsimd queue (which would
                    # stall gather descriptor generation)
                    slack = 1 if ch < c.CH - 1 else 0
                    if not done[ch] and g >= ag_group(ch) + slack:
                        done[ch] = True
                        nc.gpsimd.collective_compute(
                            "AllGather", mybir.AluOpType.bypass,
                            replica_groups=[list(range(c.NC))],
                            ins=[cc_in[layer - 1][ch][:]],
                            outs=[cc_out[layer - 1][ch][:]],
                        )
            return maybe

        # ---------------- layer 1 dense ----------------
        # x arrives pre-scaled by dinv AND pre-transposed on the host, so each
        # block's [feat, node] slice loads directly as the matmul's stationary
        # operand — no PE transpose / PSUM round-trip in the ramp.
        ag1 = emit_ag(1)
        for g in range(c.NG):
            for b in range(g * c.GRP, min((g + 1) * c.GRP, c.NB)):
                r0 = b * 128
                nrows = min(128, c.OWN - r0)
                xtb = xh.tile([128, 128], F32, tag="xh")
                if nrows < 128:
                    nc.vector.memset(xtb[:], 0.0)
                nc.sync.dma_start(out=xtb[:, :nrows], in_=xT[:, r0:r0 + nrows])
                dp = ps_d.tile([128, c.H], F32, tag="dps")
                nc.tensor.matmul(dp[:], lhsT=xtb[:], rhs=w1_sb[:], start=True, stop=True)
                u = uhl.tile([128, c.H], BF16, tag="u")
                nc.scalar.copy(out=u[:], in_=dp[:])
                write_rows(1, b, u, c.H)
            ag1(g)

        # ---------------- aggregation layers ----------------
        def agg_layer(layer):
            """Aggregate from cc_out[layer-1]; layer<3: LN+ReLU (dinv folded) then
            dense(layer+1) / table write; layer==3: swapped matmul -> @W3 -> y."""
            psum_tiles = {}
            ag_next = emit_ag(layer + 1) if layer < 3 else None
            for g in range(c.NG):
                blocks = range(g * c.GRP, min((g + 1) * c.GRP, c.NB))
                # self-loop contributions: local cc_in rows, identity matmul opens PSUM
                for b in blocks:
                    ps = ps_a.tile([128, c.H], F32, tag="aps", name=f"aps_{layer}_{b}")
                    psum_tiles[b] = ps
                    gs = sfp.tile([128, c.H], BF16, tag="gs")
                    for (ch, row0, slot0, nrows) in _block_row_segments(c, b):
                        nc.scalar.dma_start(out=gs[slot0:slot0 + nrows, :],
                                            in_=cc_in[layer - 1][ch][row0:row0 + nrows, :])
                    if layer < 3:
                        nc.tensor.matmul(ps[:], lhsT=identbf_sb[:], rhs=gs[:],
                                         start=True, stop=False)
                    else:
                        nc.tensor.matmul(ps[:], lhsT=gs[:], rhs=identbf_sb[:],
                                         start=True, stop=False)
                for ch in range(c.CH):
                    for call in calls[g][ch]:
                        q0, ncols, m0, mms = (call["q0"], call["ncols"],
                                              call["m0"], call["mms"])
                        nm = len(mms)
                        gt = gp.tile([128, c.KMAXCOL, c.H], BF16, tag="gt")
                        nc.gpsimd.dma_gather(
                            gt[:, :ncols, :], cc_out[layer - 1][ch][:],
                            idx_sb[:, q0 * 8:(q0 + ncols) * 8],
                            ncols * 128, ncols * 128, c.H,
                            single_packet=False, queue_num=ch % NQ)
                        ind = ip.tile([128, c.KMAXIND, 128], F8, tag="ind")
                        nc.scalar.dma_start(
                            out=ind[:, :nm, :],
                            in_=ind_all[:, m0 * 128:(m0 + nm) * 128].rearrange(
                                "p (n s) -> p n s", s=128))
                        for (j, mrel, b, stop) in mms:
                            if layer < 3:
                                nc.tensor.matmul(
                                    psum_tiles[b][:],
                                    lhsT=ind[:, mrel, :], rhs=gt[:, j, :],
                                    start=False, stop=stop)
                            else:
                                nc.tensor.matmul(
                                    psum_tiles[b][:],
                                    lhsT=gt[:, j, :], rhs=ind[:, mrel, :],
                                    start=False, stop=stop)
                # post-process completed blocks of this group
                for b in blocks:
                    ps = psum_tiles.pop(b)
                    if layer < 3:
                        bias_nm, gain_nm, beta_nm = (f"b{layer}r", f"g{layer}r", f"be{layer}r")
                        if bias_nm in aff_sb:
                            t = lnp.tile([128, c.H], F32, tag="t")
                            nc.scalar.activation(out=t[:], in_=ps[:],
                                                 func=mybir.ActivationFunctionType.Copy,
                                                 scale=dinv_sb[:, b:b + 1])
                            nc.vector.tensor_tensor(out=t[:], in0=t[:],
                                                    in1=aff_sb[bias_nm][:],
                                                    op=mybir.AluOpType.add)
                            z = t
                        else:
                            z = ps  # dinv scale cancels inside LN (bias == 0)
                        stats = lnp.tile([128, 6], F32, tag="stats")
                        nc.vector.bn_stats(out=stats[:], in_=z[:])
                        mv = lnp.tile([128, 2], F32, tag="mv")
                        nc.vector.bn_aggr(out=mv[:], in_=stats[:])
                        sd = lnp.tile([128, 1], F32, tag="sd")
                        nc.scalar.activation(out=sd[:], in_=mv[:, 1:2],
                                             func=mybir.ActivationFunctionType.Sqrt,
                                             bias=eps_sb[:])
                        rstd = lnp.tile([128, 1], F32, tag="rstd")
                        nc.vector.reciprocal(out=rstd[:], in_=sd[:])
                        h = xh.tile([128, c.H], F32, tag="xh")
                        if gain_nm in aff_sb or beta_nm in aff_sb:
                            nbias = lnp.tile([128, 1], F32, tag="nbias")
                            nc.vector.tensor_scalar(out=nbias[:], in0=mv[:, 0:1],
                                                    scalar1=rstd[:], scalar2=-1.0,
                                                    op0=mybir.AluOpType.mult,
                                                    op1=mybir.AluOpType.mult)
                            hn = lnp.tile([128, c.H], F32, tag="hn")
                            nc.scalar.activation(out=hn[:], in_=z[:],
                                                 func=mybir.ActivationFunctionType.Copy,
                                                 scale=rstd[:], bias=nbias[:])
                            if gain_nm in aff_sb:
                                nc.vector.tensor_tensor(out=hn[:], in0=hn[:],
                                                        in1=aff_sb[gain_nm][:],
                                                        op=mybir.AluOpType.mult)
                            if beta_nm in aff_sb:
                                nc.vector.tensor_tensor(out=hn[:], in0=hn[:],
                                                        in1=aff_sb[beta_nm][:],
                                                        op=mybir.AluOpType.add)
                            nc.scalar.activation(out=h[:], in_=hn[:],
                                                 func=mybir.ActivationFunctionType.Relu,
                                                 scale=dinv_sb[:, b:b + 1])
                        else:
                            # h = dinv_next * Relu((z - mu) * rstd):
                            # scale' = rstd*dinv, bias' = -mu*rstd*dinv
                            sc = lnp.tile([128, 1], F32, tag="sc")
                            nc.vector.tensor_tensor(out=sc[:], in0=rstd[:],
                                                    in1=dinv_sb[:, b:b + 1],
                                                    op=mybir.AluOpType.mult)
                            nb = lnp.tile([128, 1], F32, tag="nb")
                            nc.vector.tensor_scalar(out=nb[:], in0=mv[:, 0:1],
                                                    scalar1=sc[:], scalar2=-1.0,
                                                    op0=mybir.AluOpType.mult,
                                                    op1=mybir.AluOpType.mult)
                            nc.scalar.activation(out=h[:], in_=z[:],
                                                 func=mybir.ActivationFunctionType.Relu,
                                                 scale=sc[:], bias=nb[:])
                        if layer == 1:
                            dense_to_table(h, 2, b)
                        else:
                            # L3 table is h2 itself (bf16)
                            h2 = uhl.tile([128, c.H], BF16, tag="u")
                            nc.scalar.copy(out=h2[:], in_=h[:])
                            write_rows(3, b, h2, c.H)
                    else:
                        # psum holds agg3_T [feat, dst]; out = (agg3_T).T @ W3
                        aT = lnp.tile([128, c.H], F32, tag="aT")
                        nc.scalar.copy(out=aT[:], in_=ps[:])
                        op = ps_d.tile([128, c.H], F32, tag="dps")
                        nc.tensor.matmul(op[:, :c.FOUT], lhsT=aT[:], rhs=w3_sb[:],
                                         start=True, stop=True)
                        o = lnp.tile([128, c.FOUT], F32, tag="o")
                        nc.scalar.activation(out=o[:], in_=op[:, :c.FOUT],
                                             func=mybir.ActivationFunctionType.Copy,
                                             scale=dinv_sb[:, b:b + 1])
                        if "b3r" in aff_sb:
                            nc.vector.tensor_tensor(out=o[:], in0=o[:],
                                                    in1=aff_sb["b3r"][:],
                                                    op=mybir.AluOpType.add)
                        r0 = b * 128
                        nrows = min(128, c.OWN - r0)
                        nc.sync.dma_start(out=y[r0:r0 + nrows, :], in_=o[:nrows, :])
                if ag_next is not None:
                    ag_next(g)

        agg_layer(1)
        agg_layer(2)
        agg_layer(3)

        for p in (ps_a, ps_d, ps_t, lnp, ip, sfp, gp, uhl, xtp, xh, consts):
            p.release()

    nc.compile()
    return nc


# ----------------------------------------------------------------------------
# Entry points
# ----------------------------------------------------------------------------
_cache = {}


def _prepare(cfg, inputs):
    c = cfg
    key = hash((np.asarray(inputs["edge_index"]).tobytes(),
                np.asarray(inputs["x"]).tobytes()))
    if key in _cache:
        return _cache[key]

    meta, dinv, idx_all, ind_all = _preprocess(c, inputs["edge_index"])

    trivial = {
        "b1r": not np.any(inputs["b1"]), "g1r": bool(np.all(inputs["g1"] == 1.0)),
        "be1r": not np.any(inputs["be1"]), "b2r": not np.any(inputs["b2"]),
        "g2r": bool(np.all(inputs["g2"] == 1.0)), "be2r": not np.any(inputs["be2"]),
        "b3r": not np.any(inputs["b3"]),
    }
    nc = _build_program(c, meta, trivial)

    shared = {
        "w1": np.asarray(inputs["W1"], dtype=np.float32),
        "w2": np.asarray(inputs["W2"], dtype=np.float32),
        "w3": np.asarray(inputs["W3"], dtype=np.float32),
        "ident": np.eye(128, dtype=np.float32),
        "ident_bf": np.eye(128, dtype=np.float32).astype(ml_dtypes.bfloat16),
    }
    for nm, src in (("b1r", "b1"), ("g1r", "g1"), ("be1r", "be1"), ("b2r", "b2"),
                    ("g2r", "g2"), ("be2r", "be2"), ("b3r", "b3")):
        if not trivial[nm]:
            shared[nm] = np.asarray(inputs[src], dtype=np.float32)[None, :].repeat(128, 0).copy()

    x_np = np.asarray(inputs["x"], dtype=np.float32)
    in_maps = []
    for k in range(c.NC):
        dv = dinv[k * c.OWN:(k + 1) * c.OWN]
        dcols = np.zeros((128, c.NB), dtype=np.float32)
        npad = c.NB * 128 - c.OWN
        dvp = np.concatenate([dv, np.ones(npad, dtype=np.float32)])
        dcols[:, :] = dvp.reshape(c.NB, 128).T
        m = dict(shared)
        # pre-scale x rows by dinv and pre-transpose on the host:
        # u1 = (dinv*x)@W1, loaded block-wise as the stationary operand
        m["xT"] = np.ascontiguousarray(
            (x_np[k * c.OWN:(k + 1) * c.OWN] * dv[:, None]).T)
        m["idx_all"] = np.tile(idx_all[k], (8, 1))
        m["ind_all"] = ind_all[k]
        m["dinv_cols"] = dcols
        in_maps.append(m)

    _cache[key] = (nc, in_maps)
    return nc, in_maps


def _run(cfg, inputs, trace=False):
    nc, in_maps = _prepare(cfg, inputs)
    res = bass_utils.run_bass_kernel_spmd(
        nc, in_maps, core_ids=list(range(cfg.NC)), trace=trace)
    out = np.concatenate([res.results[k]["y"] for k in range(cfg.NC)], axis=0)
    return out, res


def kernel(**inputs):
    cfg = Cfg()
    out, _ = _run(cfg, inputs)
    return out
